# revision 9
# baseline (speedup 1.0000x reference)
"""CNN-LSTM Trainium2 kernel (nn_CNN_LSTM_41205916238256), v2.

Pipeline per core (batch-parallel, 32 batch elems per core):
  1. Embedding gather+transpose fused: per-core COMPACT table (unique
     tokens, <=16384 rows -> int16 ids) + one dma_gather(transpose=True)
     per batch elem -> embT_all[e_l, (b*2+eh)*512 + tok] directly
     (SWDGE descriptor gen on GpSimd, transfer on DMA engines).
  3. Conv(width 5, full E contraction) in 2 t-slices ([0,252), [252,508)):
     10 accumulated matmuls per (b, nf-half); ReLU+bias on ACT ->
     convT[nh][p, b*256 + t_local] bf16 (contiguous writes).
  4. Input projection Xp = W_ih_eff @ relu_conv + bias_eff in 4 t-blocks
     ([0,127),[127,252),[252,380),[380,508)) of 16-t sub-slices;
     stored [128, loc*128 + g*32 + b] bf16 in 2 rotating block tiles.
     Conv slice 1 and Xp blocks 1-3 are DRIP-FED between recurrence steps
     (a few PE instrs + <=1 ACT instr per step) so the recurrence starts
     after only conv slice 0 + the first Xp sub-slice.
  5. 508-step LSTM recurrence, transposed layout (H on partitions):
       bank_t = Xp_t (identity inject, early) + U_eff @ r_{t-1} (4 MMs)
       S = sigmoid(bank)          [f,i,g,o at cols 0/32/64/96], ONE 128-col ACT
       t1 = (S_g - .5) * S_i ; t2 = S_f * P ; P' = t1 + t2
       sigP = tanh(2*P') ; r = sigP * S_o   (bf16)
     with P == c-scaled, r == h/2; prescales folded into weights on host:
       W_ih/bias rows: g x2;  w_hh rows: f,i,o x2, g x4.
  6. h_n = sigP * S_o in fp32, DMA out transposed [128, 32].
"""
import numpy as np
import ml_dtypes

import concourse.bacc as bacc
import concourse.bass as bass
import concourse.mybir as mybir
import concourse.tile as tile
from concourse.bass_utils import run_bass_kernel_spmd

BF16 = mybir.dt.bfloat16
F32 = mybir.dt.float32
I32 = mybir.dt.int32
I16 = mybir.dt.int16
AF = mybir.ActivationFunctionType
OP = mybir.AluOpType

VOCAB, EMB, KER, NF, HID = 50257, 256, 5, 256, 128
B, S = 256, 512
T = S - KER + 1            # 508
NC = 8                     # cores
BL = B // NC               # 32 batch per core
P = 128
TQ = S // P                # 4 128-token groups

# conv t-slices (slice 0 windows only touch tokens < 256 -> q0+q1)
CSL = [(0, 252), (252, 508)]
CW_T = 256                 # convT cols per b (max slice width)
# xp t-blocks (block must lie within one conv slice)
XBL = [(0, 127), (127, 252), (252, 380), (380, 508)]
XSUB = 16                  # t sub-slice for xp PSUM/copies

_PROGRAM = None


def _build_program(debug=False):
    nc = bacc.Bacc("TRN2", target_bir_lowering=False, debug=False)

    emb_d = nc.dram_tensor("embt", [BL * S, EMB], BF16, kind="ExternalInput")
    idx_d = nc.dram_tensor("idx16", [P, BL * (S // 16)], I16,
                           kind="ExternalInput")
    cw_d = nc.dram_tensor("cw", [KER * 2 * 2, P, P], BF16, kind="ExternalInput")
    cb_d = nc.dram_tensor("cb", [P, 2], F32, kind="ExternalInput")
    wih_d = nc.dram_tensor("wih", [4 * 2, P, P], BF16, kind="ExternalInput")
    be_d = nc.dram_tensor("be", [P, 4], F32, kind="ExternalInput")
    u_d = nc.dram_tensor("u", [4, P, P], BF16, kind="ExternalInput")
    eye_d = nc.dram_tensor("eye", [P, P], BF16, kind="ExternalInput")
    r0_d = nc.dram_tensor("r0", [P, BL], BF16, kind="ExternalInput")
    hT_d = nc.dram_tensor("hT", [P, BL], F32, kind="ExternalOutput")
    if debug:
        embT_dump = nc.dram_tensor("embT_dump", [P, 2 * BL * S], BF16,
                                   kind="ExternalOutput")
        convT_dump = nc.dram_tensor("convT_dump", [2, P, BL * CW_T], BF16,
                                    kind="ExternalOutput")
        xp_dump = nc.dram_tensor("xp_dump", [P, 128 * P], BF16,
                                 kind="ExternalOutput")

    with tile.TileContext(nc) as tc:
        with tc.tile_pool(name="stat", bufs=1) as stat:
            # ---- static loads
            idx_t = stat.tile([P, BL * (S // 16)], I16, tag="idx")
            nc.sync.dma_start(out=idx_t[:], in_=idx_d[:])
            cw_t = []
            for k in range(KER):
                for eh in range(2):
                    for nh in range(2):
                        w = stat.tile([P, P], BF16, tag=f"cw{k}{eh}{nh}")
                        nc.sync.dma_start(out=w[:], in_=cw_d[(k * 2 + eh) * 2 + nh])
                        cw_t.append(w)
            cwf = lambda k, eh, nh: cw_t[(k * 2 + eh) * 2 + nh]
            cb_t = stat.tile([P, 2], F32, tag="cb")
            nc.sync.dma_start(out=cb_t[:], in_=cb_d[:])
            wih_t = []
            for g in range(4):
                for kh in range(2):
                    w = stat.tile([P, P], BF16, tag=f"wih{g}{kh}")
                    nc.sync.dma_start(out=w[:], in_=wih_d[g * 2 + kh])
                    wih_t.append(w)
            be_t = stat.tile([P, 4], F32, tag="be")
            nc.sync.dma_start(out=be_t[:], in_=be_d[:])
            u_t = []
            for g in range(4):
                w = stat.tile([P, P], BF16, tag=f"u{g}")
                nc.sync.dma_start(out=w[:], in_=u_d[g])
                u_t.append(w)
            eye_t = stat.tile([P, P], BF16, tag="eye")
            nc.sync.dma_start(out=eye_t[:], in_=eye_d[:])
            r0_t = stat.tile([P, BL], BF16, tag="r0")
            nc.sync.dma_start(out=r0_t[:], in_=r0_d[:])

            # big SBUF tensors
            # embT_all[e_l, (b*2+eh)*512 + tok]
            embT = stat.tile([P, 2 * BL * S], BF16, tag="embT")
            # convT[nh][p, b*256 + t_local] for the current conv slice
            convT = [stat.tile([P, BL * CW_T], BF16, tag=f"convT{nh}",
                               name=f"convT{nh}") for nh in range(2)]
            # xp block tiles, rotating: col = loc*128 + g*32 + b
            xp_t = [stat.tile([P, 128 * P], BF16, tag=f"xp{i}", name=f"xp{i}")
                    for i in range(2)]
            # ============ GATHER+TRANSPOSE (fused, one call per b) ======
            # embT_v[p, blk, tok]; blk = b*2+eh
            embT_v = embT[:].rearrange("p (blk s) -> p blk s", blk=2 * BL, s=S)
            SW = S // 16           # idx cols per batch elem
            for b in range(BL):
                nc.gpsimd.dma_gather(
                    out_ap=embT_v[:, 2 * b:2 * b + 2, :],
                    in_ap=emb_d[:],
                    idxs_ap=idx_t[:, b * SW:(b + 1) * SW],
                    num_idxs=S, num_idxs_reg=S, elem_size=EMB,
                    transpose=True)
            if debug:
                nc.sync.dma_start(out=embT_dump[:], in_=embT[:])

            # ================= helpers =================
            with tc.tile_pool(name="pcps", bufs=2, space="PSUM") as pcps, \
                 tc.tile_pool(name="pxps", bufs=2, space="PSUM") as pxps, \
                 tc.tile_pool(name="rps", bufs=3, space="PSUM") as rps, \
                 tc.tile_pool(name="rdyn", bufs=3) as dyn:

                def conv_group(sl, b, nh):
                    """Emit 10 MM thunks + 1 relu thunk for (slice, b, nh)."""
                    t0, t1 = CSL[sl]
                    L = t1 - t0
                    mms = []
                    state = {}

                    def start():
                        state["cps"] = pcps.tile([P, CW_T], F32, tag="cps", name="cps")
                    n_mm = 0
                    for k in range(KER):
                        for eh in range(2):
                            def mm(k=k, eh=eh, n_mm=n_mm, b=b, nh=nh, t0=t0, L=L):
                                if n_mm == 0:
                                    start()
                                base = (b * 2 + eh) * S + t0 + k
                                nc.tensor.matmul(
                                    out=state["cps"][:, 0:L],
                                    lhsT=cwf(k, eh, nh)[:],
                                    rhs=embT[:, base:base + L],
                                    start=(n_mm == 0), stop=(n_mm == 9))
                            mms.append(mm)
                            n_mm += 1

                    def relu(b=b, nh=nh, L=L):
                        nc.scalar.activation(
                            convT[nh][:, b * CW_T:b * CW_T + L],
                            state["cps"][:, 0:L], AF.Relu,
                            bias=cb_t[:, nh:nh + 1])
                    return mms, relu

                # convT viewed [p, b, t_local] -> rhs needs (t outer, b inner)
                convT_tb = [convT[kh][:].rearrange(
                    "p (b t) -> p t b", b=BL, t=CW_T) for kh in range(2)]

                def xp_sub(blk, ts, L, g):
                    """Emit 2 MM thunks + 1 copy thunk for xp sub-slice."""
                    t0 = XBL[blk][0]
                    xpt = xp_t[blk % 2]
                    sl = 0 if t0 < CSL[1][0] else 1
                    cs = CSL[sl][0]
                    state = {}
                    mms = []
                    for kh in range(2):
                        def mm(kh=kh, ts=ts, L=L, g=g, cs=cs, sl=sl):
                            if kh == 0:
                                state["xps"] = pxps.tile([P, XSUB * BL], F32,
                                                         tag="xps", name="xps")
                            nc.tensor.matmul(
                                out=state["xps"][:, 0:L * BL],
                                lhsT=wih_t[g * 2 + kh][:],
                                rhs=convT_tb[kh][:, ts - cs:ts - cs + L, :],
                                start=(kh == 0), stop=(kh == 1))
                        mms.append(mm)

                    def copy(blk=blk, ts=ts, L=L, g=g, t0=t0, xpt=xpt):
                        dst = xpt[:].rearrange(
                            "p (t g b) -> p t g b", t=128, g=4, b=BL)[
                            :, ts - t0:ts - t0 + L, g, :]
                        nc.scalar.activation(dst, state["xps"][:, 0:L * BL],
                                             AF.Identity, bias=be_t[:, g:g + 1])
                    return mms, copy

                def xp_block_thunks(blk):
                    t0, t1 = XBL[blk]
                    pe_q, act_q = [], []
                    ts = t0
                    while ts < t1:
                        L = min(XSUB, t1 - ts)
                        for g in range(4):
                            mms, copy = xp_sub(blk, ts, L, g)
                            pe_q.extend(mms)
                            act_q.append((len(pe_q), copy))
                        ts += L
                    return pe_q, act_q

                def conv_slice_thunks(sl):
                    pe_q, act_q = [], []
                    for b in range(BL):
                        for nh in range(2):
                            mms, relu = conv_group(sl, b, nh)
                            pe_q.extend(mms)
                            act_q.append((len(pe_q), relu))
                    return pe_q, act_q

                # ======== PRE-RECURRENCE: conv slice 0 + xp block 0 ========
                pe0, act0 = conv_slice_thunks(0)
                i_a = 0
                for i_p, th in enumerate(pe0):
                    th()
                    while i_a < len(act0) and act0[i_a][0] <= i_p + 1:
                        act0[i_a][1](); i_a += 1
                assert i_a == len(act0)
                peb, actb = xp_block_thunks(0)
                i_a = 0
                for i_p, th in enumerate(peb):
                    th()
                    while i_a < len(actb) and actb[i_a][0] <= i_p + 1:
                        actb[i_a][1](); i_a += 1
                assert i_a == len(actb)
                if debug:
                    for nh in range(2):
                        nc.sync.dma_start(out=convT_dump[nh], in_=convT[nh][:])
                    nc.sync.dma_start(out=xp_dump[:], in_=xp_t[0][:])

                # ======== drip phases ========
                pA_pe, pA_act = xp_block_thunks(1)
                c1_pe, c1_act = conv_slice_thunks(1)
                off = len(pA_pe)
                pA_pe = pA_pe + c1_pe
                pA_act = pA_act + [(pre + off, th) for pre, th in c1_act]
                pB_pe, pB_act = xp_block_thunks(2)
                pC_pe, pC_act = xp_block_thunks(3)
                phases = [
                    (0, {"pe": pA_pe, "act": pA_act, "ip": 0, "ia": 0}),
                    (XBL[1][0], {"pe": pB_pe, "act": pB_act, "ip": 0, "ia": 0}),
                    (XBL[2][0], {"pe": pC_pe, "act": pC_act, "ip": 0, "ia": 0}),
                ]

                def drip(t, pe_budget=7, act_budget=1):
                    for start_t, ph in phases:
                        if t < start_t:
                            break
                        if ph["ip"] < len(ph["pe"]) or ph["ia"] < len(ph["act"]):
                            while pe_budget > 0 and ph["ip"] < len(ph["pe"]):
                                ph["pe"][ph["ip"]](); ph["ip"] += 1
                                pe_budget -= 1
                            done = ph["ip"] >= len(ph["pe"])
                            while (act_budget > 0 and ph["ia"] < len(ph["act"])
                                   and (done or
                                        ph["act"][ph["ia"]][0] <= ph["ip"] - 8)):
                                ph["act"][ph["ia"]][1](); ph["ia"] += 1
                                act_budget -= 1
                            break

                # ================= RECURRENCE =================
                P_prev = stat.tile([P, BL], F32, tag="P_init")
                nc.vector.memset(P_prev[:], 0.0)
                r_prev = r0_t
                S_t = None
                sigP = None
                for t in range(T):
                    blk = next(i for i, (a, bnd) in enumerate(XBL) if t < bnd)
                    loc = t - XBL[blk][0]
                    xpt = xp_t[blk % 2]
                    bank = rps.tile([P, P], F32, tag="bank")
                    nc.tensor.matmul(out=bank[:], lhsT=eye_t[:],
                                     rhs=xpt[:, loc * P:(loc + 1) * P],
                                     start=True, stop=False)
                    for g in range(4):
                        nc.tensor.matmul(out=bank[:, g * BL:(g + 1) * BL],
                                         lhsT=u_t[g][:], rhs=r_prev[:],
                                         start=False, stop=True)
                    drip(t)
                    S_t = dyn.tile([P, P], F32, tag="S")
                    nc.scalar.activation(S_t[:], bank[:], AF.Sigmoid)
                    t1 = dyn.tile([P, BL], F32, tag="t1")
                    nc.vector.scalar_tensor_tensor(
                        out=t1[:], in0=S_t[:, 64:96], scalar=0.5, in1=S_t[:, 32:64],
                        op0=OP.subtract, op1=OP.mult)
                    t2 = dyn.tile([P, BL], F32, tag="t2")
                    nc.vector.tensor_tensor(out=t2[:], in0=S_t[:, 0:32],
                                            in1=P_prev[:], op=OP.mult)
                    P_new = dyn.tile([P, BL], F32, tag="Pn")
                    nc.vector.tensor_tensor(out=P_new[:], in0=t1[:], in1=t2[:],
                                            op=OP.add)
                    sigP = dyn.tile([P, BL], F32, tag="sigP")
                    nc.scalar.activation(sigP[:], P_new[:], AF.Tanh, scale=2.0)
                    r_new = dyn.tile([P, BL], BF16, tag="r")
                    nc.vector.tensor_tensor(out=r_new[:], in0=sigP[:],
                                            in1=S_t[:, 96:128], op=OP.mult)
                    r_prev, P_prev = r_new, P_new

                for _, ph in phases:
                    assert ph["ip"] == len(ph["pe"]) and ph["ia"] == len(ph["act"])

                # exact final h = tanh(c) * sigma(o) in fp32
                hT = dyn.tile([P, BL], F32, tag="hT")
                nc.vector.tensor_tensor(out=hT[:], in0=sigP[:],
                                        in1=S_t[:, 96:128], op=OP.mult)
                nc.sync.dma_start(out=hT_d[:], in_=hT[:])

    nc.compile()
    return nc


def _prep_inputs(text, h_0, emb, conv_w, conv_b, w_ih, w_hh, b_ih, b_hh):
    bf = ml_dtypes.bfloat16
    text = np.asarray(text)
    h_0 = np.asarray(h_0, dtype=np.float32)
    emb = np.asarray(emb, dtype=np.float32)
    conv_w = np.asarray(conv_w, dtype=np.float32)
    conv_b = np.asarray(conv_b, dtype=np.float32)
    w_ih = np.asarray(w_ih, dtype=np.float32)
    w_hh = np.asarray(w_hh, dtype=np.float32)
    b_ih = np.asarray(b_ih, dtype=np.float32)
    b_hh = np.asarray(b_hh, dtype=np.float32)

    emb_bf = np.ascontiguousarray(emb.astype(bf))

    # conv weights: cw[k,eh,nh][e,n] = conv_w[nh*128+n, 0, k, eh*128+e]
    cw = conv_w[:, 0, :, :]                       # [NF, KER, EMB]
    cw = cw.transpose(1, 2, 0)                    # [KER, EMB, NF]
    cw = cw.reshape(KER, 2, P, 2, P)              # k, eh, e, nh, n
    cw = cw.transpose(0, 1, 3, 2, 4)              # k, eh, nh, e, n
    cw_in = np.ascontiguousarray(cw.reshape(KER * 4, P, P).astype(bf))
    cb_in = np.ascontiguousarray(conv_b.reshape(2, P).T)

    # gate reorder torch [i,f,g,o] -> ours [f,i,g,o]
    perm = [1, 0, 2, 3]
    wih_g = w_ih.reshape(4, P, NF)[perm]          # [4, 128, NF]
    whh_g = w_hh.reshape(4, P, HID)[perm]
    bias_g = (b_ih + b_hh).reshape(4, P)[perm]
    wih_g = wih_g * np.array([1, 1, 2, 1], np.float32)[:, None, None]
    bias_g = bias_g * np.array([1, 1, 2, 1], np.float32)[:, None]
    whh_g = whh_g * np.array([1, 1, 2, 1], np.float32)[:, None, None]

    # wih lhsT tiles: [g,kh][k,m] = wih_g[g, m, kh*128+k]
    wih_in = np.ascontiguousarray(
        wih_g.reshape(4, P, 2, P).transpose(0, 2, 3, 1)
        .reshape(8, P, P).astype(bf))
    be_in = np.ascontiguousarray(bias_g.reshape(4, P).T)
    # u lhsT tiles: [g][k,m] = whh_g[g, m, k]
    u_in = np.ascontiguousarray(whh_g.transpose(0, 2, 1).astype(bf))
    eye_in = np.eye(P, dtype=np.float32).astype(bf)

    text32 = text.astype(np.int64)
    in_maps = []
    for cidx in range(NC):
        tloc = text32[cidx * BL:(cidx + 1) * BL]           # [BL, S]
        # compact per-core table: unique tokens -> int16 ids
        uniq, inv = np.unique(tloc, return_inverse=True)
        assert len(uniq) <= BL * S
        emb_small = np.zeros((BL * S, EMB), emb_bf.dtype)
        emb_small[:len(uniq)] = emb_bf[uniq]
        cid = inv.reshape(BL, S).astype(np.int16)
        # idx16[p, b*(S//16)+s] = cid[b, s*16+p%16], replicated 8x over
        # the 128 partitions (one copy per DMA channel group)
        idx16 = np.ascontiguousarray(np.tile(
            cid.reshape(BL, S // 16, 16).transpose(2, 0, 1)
            .reshape(16, BL * (S // 16)), (8, 1)))
        r0 = np.ascontiguousarray(
            h_0[0, cidx * BL:(cidx + 1) * BL].T.astype(bf))
        in_maps.append({
            "embt": emb_small, "idx16": idx16, "cw": cw_in, "cb": cb_in,
            "wih": wih_in, "be": be_in, "u": u_in, "eye": eye_in, "r0": r0,
        })
    return in_maps


def kernel(**inputs) -> np.ndarray:
    global _PROGRAM
    if _PROGRAM is None:
        _PROGRAM = _build_program()
    in_maps = _prep_inputs(**inputs)
    res = run_bass_kernel_spmd(_PROGRAM, in_maps, core_ids=list(range(NC)))
    out = np.empty((B, HID), np.float32)
    for cidx in range(NC):
        out[cidx * BL:(cidx + 1) * BL] = res.results[cidx]["hT"].T
    return out
